# revision 9
# baseline (speedup 1.0000x reference)
"""Trainium2 Bass kernel for causal multi-head attention (B=2, S=2048, D=1024, 16 heads x 64).

Sharding: 8 cores = 2 batches x 4 head-groups (tensor parallel over heads).
Each core computes 4 heads of one batch, projects through W_O, and the
4 cores of a batch ReduceScatter-sum their partial outputs (each core ends
up with a disjoint 512-row shard of the summed output). Host concatenates.

Attention is computed flash-style with transposed scores:
  sT[k, q] = (K Q^T) so softmax sum over k is the partition dim; the
  denominator is produced for free by appending a ones-column to the AV
  stationary operand ([v | 1] -> out rows = [attn_outT | d]).
Causality is exploited at tile granularity (upper-triangular tiles skipped,
diagonal tiles masked post-exp).
"""

import os
import sys

sys.path.insert(0, "/opt/trn_rl_repo")

import numpy as np

# ---- problem constants (hardcoded; kernel.py must be self-contained) ----
B = 2
S = 2048
D = 1024
N_HEADS = 16
DH = 64                 # head dim
NCORES = 8
NH_CORE = N_HEADS // 4  # 4 heads per core (4-way TP x 2-way batch DP)
SCALE = 1.0 / 8.0       # 1/sqrt(64)

P = 128                 # partitions
DC = D // P             # 8 contraction chunks for the projections
KC = S // P             # 16 key chunks
QT = 512                # q tile width (free dim) for scores / AV
NQT = S // QT           # 4 q tiles
NT = 512                # moving-operand tile for projections / out-proj

_CACHE = {}


def _build():
    import concourse.bass as bass
    import concourse.tile as tile
    from concourse import bacc, mybir

    f32 = mybir.dt.float32
    MM = mybir.dt.float32r  # matmul dtype: fp32r = full-rate reduced-precision fp32

    nc = bacc.Bacc(
        "TRN2",
        target_bir_lowering=False,
        debug=False,
        enable_asserts=False,
        num_devices=NCORES,
    )

    xt_d = nc.dram_tensor("xt", [D, S], MM, kind="ExternalInput").ap()
    wqt_d = nc.dram_tensor("wqt", [D, NH_CORE * DH], MM, kind="ExternalInput").ap()
    wkt_d = nc.dram_tensor("wkt", [D, NH_CORE * DH], MM, kind="ExternalInput").ap()
    wvt_d = nc.dram_tensor("wvt", [D, NH_CORE * DH], MM, kind="ExternalInput").ap()
    wof_d = nc.dram_tensor("wof", [NH_CORE * DH, D], MM, kind="ExternalInput").ap()
    msk_d = nc.dram_tensor("msk", [4 * P, QT], MM, kind="ExternalInput").ap()
    out_d = nc.dram_tensor("out", [S // 4, D], f32, kind="ExternalOutput").ap()

    Exp = mybir.ActivationFunctionType.Exp

    with tile.TileContext(nc) as tc:
        with (
            tc.tile_pool(name="const", bufs=1) as const,
            tc.tile_pool(name="work", bufs=2) as work,
            tc.tile_pool(name="ps", bufs=2, space="PSUM") as ps_pool,
            tc.tile_pool(name="attnps", bufs=1, space="PSUM") as attnps_pool,
            tc.tile_pool(name="dram", bufs=1, space="DRAM") as dram,
        ):
            # ---------------- input DMAs ----------------
            wq_sb = const.tile([P, DC, NH_CORE * DH], MM)
            nc.sync.dma_start(wq_sb[:], wqt_d.rearrange("(c p) n -> p c n", p=P))
            wk_sb = const.tile([P, DC, NH_CORE * DH], MM)
            nc.sync.dma_start(wk_sb[:], wkt_d.rearrange("(c p) n -> p c n", p=P))
            wv_sb = const.tile([P, DC, NH_CORE * DH], MM)
            nc.sync.dma_start(wv_sb[:], wvt_d.rearrange("(c p) n -> p c n", p=P))

            # residual^T, split per d-chunk so QKV matmuls can start early
            xt_sb = const.tile([P, DC, S], MM)
            xt_r = xt_d.rearrange("(c p) s -> p c s", p=P)
            for dc in range(DC):
                nc.sync.dma_start(xt_sb[:, dc, :], xt_r[:, dc, :])

            wo_sb = const.tile([P, 2, D], MM)
            nc.sync.dma_start(wo_sb[:], wof_d.rearrange("(c p) d -> p c d", p=P))
            mask_sb = const.tile([P, 4, QT], MM)
            nc.sync.dma_start(mask_sb[:], msk_d.rearrange("(m p) q -> p m q", p=P))

            # ---------------- QKV projections ----------------
            # qT/kT: per head-pair tiles [128 = 2 heads x 64, S]
            qT = [const.tile([P, S], MM, name=f"qT{i}") for i in range(2)]
            kT = [const.tile([P, S], MM, name=f"kT{i}") for i in range(2)]
            for pr in range(2):
                for w_sb, dst in ((wq_sb, qT[pr]), (wk_sb, kT[pr])):
                    for ntile in range(S // NT):
                        pp = ps_pool.tile([P, NT], f32, name="work_ps")
                        for dc in range(DC):
                            nc.tensor.matmul(
                                pp[:],
                                w_sb[:, dc, pr * P : (pr + 1) * P],
                                xt_sb[:, dc, ntile * NT : (ntile + 1) * NT],
                                start=(dc == 0),
                                stop=(dc == DC - 1),
                            )
                        nc.scalar.copy(dst[:, ntile * NT : (ntile + 1) * NT], pp[:])

            # v in natural [k, h] layout, augmented with a ones column:
            # AV stationary [v | 1] makes out row 64 the softmax denominator.
            v_aug = [const.tile([P, KC, DH + 1], MM, name=f"vaug{h}") for h in range(NH_CORE)]
            ones_f32 = const.tile([P, DH], f32)
            nc.vector.memset(ones_f32[:], 1.0)
            ones_sb = const.tile([P, DH], MM)
            nc.scalar.copy(ones_sb[:], ones_f32[:])
            for h in range(NH_CORE):
                nc.scalar.copy(v_aug[h][:, :, DH : DH + 1], ones_f32[:, 0:KC, None])
            for pc in range(KC):
                vp = ps_pool.tile([P, NH_CORE * DH], f32, name="work_ps")
                for dc in range(DC):
                    nc.tensor.matmul(
                        vp[:],
                        xt_sb[:, dc, pc * P : (pc + 1) * P],
                        wv_sb[:, dc, :],
                        start=(dc == 0),
                        stop=(dc == DC - 1),
                    )
                for h in range(NH_CORE):
                    nc.vector.tensor_copy(
                        v_aug[h][:, pc, 0:DH],
                        vp[:, h * DH : (h + 1) * DH],
                    )

            # ---------------- attention ----------------
            attnT = [const.tile([P, S], MM, name=f"attnT{i}") for i in range(2)]
            for pr in range(2):
                for qt in range(NQT):
                    q_sl = slice(qt * QT, (qt + 1) * QT)
                    nk = (qt + 1) * (QT // P)  # causal: k chunks 0..nk-1
                    attn_ps = [
                        attnps_pool.tile([P, QT], f32, name=f"attn{h2}") for h2 in range(2)
                    ]
                    for kb in range(nk):
                        k_sl = slice(kb * P, (kb + 1) * P)
                        s_ps = [
                            ps_pool.tile([P, QT], f32, name=f"s{h2}", bufs=2)
                            for h2 in range(2)
                        ]
                        for h2 in range(2):
                            hb = h2 * DH  # partition base of this head in the pair tiles
                            nc.tensor.matmul(
                                s_ps[h2][:],
                                kT[pr][hb : hb + DH, k_sl],
                                qT[pr][hb : hb + DH, q_sl],
                                start=True,
                                stop=True,
                            )
                        for h2 in range(2):
                            h = pr * 2 + h2
                            pat = work.tile([P, QT], MM, name="pat", bufs=3)
                            nc.scalar.activation(pat[:], s_ps[h2][:], Exp, scale=SCALE)
                            if kb >= qt * (QT // P):  # diagonal tile: causal mask
                                ri = kb - qt * (QT // P)
                                nc.vector.tensor_mul(pat[:], pat[:], mask_sb[:, ri, :])
                            nc.tensor.matmul(
                                attn_ps[h2][0 : DH + 1, :],
                                v_aug[h][:, kb, :],
                                pat[:],
                                start=(kb == 0),
                                stop=(kb == nk - 1),
                            )
                    # normalize: recip of d (row 64, lane-aligned), broadcast
                    # across partitions via a K=1 PE outer product, multiply.
                    for h2 in range(2):
                        recip = work.tile([P, QT], MM, name="recip", bufs=2)
                        with nc.allow_low_precision(reason="fp32r recip feeds fp32r matmul"):
                            nc.vector.reciprocal(
                                recip[DH : DH + 1, :], attn_ps[h2][DH : DH + 1, :]
                            )
                        rb_ps = ps_pool.tile([DH, QT], f32, name="work_ps")
                        nc.tensor.matmul(
                            rb_ps[:],
                            ones_sb[DH : DH + 1, :],
                            recip[DH : DH + 1, :],
                            start=True,
                            stop=True,
                        )
                        # bounce broadcast to SBUF: DVE TT cannot read 2 PSUM operands
                        rb_sb = work.tile([DH, QT], f32, name="rb_sb", bufs=2)
                        nc.scalar.copy(rb_sb[:], rb_ps[:])
                        if h2 == 0:
                            nc.vector.tensor_mul(
                                attnT[pr][0:DH, q_sl], attn_ps[0][0:DH, :], rb_sb[:]
                            )
                        else:
                            u_b = work.tile([DH, QT], MM, name="u_b", bufs=2)
                            nc.vector.tensor_mul(u_b[:], attn_ps[1][0:DH, :], rb_sb[:])
                            nc.sync.dma_start(attnT[pr][DH:P, q_sl], u_b[:])

            # ---------------- output projection + collective ----------------
            cc_in = dram.tile([S, D], f32)
            cc_out = dram.tile([S // 4, D], f32)
            for pc in range(KC):
                osb = work.tile([P, D], f32, name="osb", bufs=2)
                for dt_ in range(D // NT):
                    op = ps_pool.tile([P, NT], f32, name="work_ps")
                    for pr in range(2):
                        nc.tensor.matmul(
                            op[:],
                            attnT[pr][:, pc * P : (pc + 1) * P],
                            wo_sb[:, pr, dt_ * NT : (dt_ + 1) * NT],
                            start=(pr == 0),
                            stop=(pr == 1),
                        )
                    nc.scalar.copy(osb[:, dt_ * NT : (dt_ + 1) * NT], op[:])
                nc.sync.dma_start(cc_in[pc * P : (pc + 1) * P, :], osb[:])

            nc.gpsimd.collective_compute(
                "ReduceScatter",
                mybir.AluOpType.add,
                replica_groups=[[0, 1, 2, 3], [4, 5, 6, 7]],
                ins=[cc_in.opt()],
                outs=[cc_out.opt()],
            )
            nc.sync.dma_start(out_d, cc_out[:])

    nc.compile()
    return nc


def _get_nc():
    if "nc" not in _CACHE:
        _CACHE["nc"] = _build()
    return _CACHE["nc"]


def _masks():
    m = np.zeros((4, P, QT), np.float32)
    k = np.arange(P)[:, None]
    q = np.arange(QT)[None, :]
    for r in range(4):
        m[r] = (q >= r * P + k).astype(np.float32)
    return m.reshape(4 * P, QT)


def _ensure_ntff_hook():
    """Register the axon NTFF profile hook (missing antenv.axon_hooks shim)."""
    import types

    try:
        from antenv.axon_hooks import get_axon_ntff_profile_hook  # noqa: F401

        return
    except ImportError:
        pass
    import antenv

    if "/root/.axon_site" not in sys.path:
        sys.path.insert(0, "/root/.axon_site")
    from trn_agent_boot.trn_boot import _ntff_profile_via_ctypes

    hook = _ntff_profile_via_ctypes("/opt/axon/libaxon_pjrt.so")
    mod = types.ModuleType("antenv.axon_hooks")
    mod.get_axon_ntff_profile_hook = lambda: hook
    mod.set_axon_ntff_profile_hook = lambda h: None
    sys.modules["antenv.axon_hooks"] = mod
    antenv.axon_hooks = mod


def kernel(residual, W_Q, W_K, W_V, W_O):
    from concourse.bass_utils import run_bass_kernel_spmd

    if int(os.environ.get("KERNEL_TRACE", "0")):
        _ensure_ntff_hook()

    residual = np.ascontiguousarray(np.asarray(residual), np.float32)
    W_Q = np.ascontiguousarray(np.asarray(W_Q), np.float32)
    W_K = np.ascontiguousarray(np.asarray(W_K), np.float32)
    W_V = np.ascontiguousarray(np.asarray(W_V), np.float32)
    W_O = np.ascontiguousarray(np.asarray(W_O), np.float32)

    nc = _get_nc()
    msk = _masks()
    in_maps = []
    for c in range(NCORES):
        b, g = divmod(c, 4)
        hs = slice(g * NH_CORE, (g + 1) * NH_CORE)
        in_maps.append(
            {
                "xt": np.ascontiguousarray(residual[b].T),
                "wqt": np.ascontiguousarray(
                    W_Q[hs].transpose(2, 0, 1).reshape(D, NH_CORE * DH)
                ),
                "wkt": np.ascontiguousarray(
                    W_K[hs].transpose(2, 0, 1).reshape(D, NH_CORE * DH)
                ),
                "wvt": np.ascontiguousarray(
                    W_V[hs].transpose(2, 0, 1).reshape(D, NH_CORE * DH)
                ),
                "wof": np.ascontiguousarray(W_O[hs].reshape(NH_CORE * DH, D)),
                "msk": msk,
            }
        )

    res = run_bass_kernel_spmd(
        nc,
        in_maps,
        core_ids=list(range(NCORES)),
        trace=bool(int(os.environ.get("KERNEL_TRACE", "0"))),
        trace_cores=[0] if int(os.environ.get("KERNEL_TRACE", "0")) else None,
    )
    _CACHE["last_results"] = res

    out = np.empty((B, S, D), np.float32)
    for b in range(B):
        out[b] = np.concatenate([res.results[b * 4 + r]["out"] for r in range(4)], axis=0)
    return out


# revision 21
# speedup vs baseline: 1.1399x; 1.1399x over previous
"""Trainium2 Bass kernel for causal multi-head attention (B=2, S=2048, D=1024, 16 heads x 64).

Sharding: 8 cores = 2 batches x 4 head-groups (tensor parallel over heads).
Each core computes attention for its 4 heads; the 4 cores of a batch then
AllToAll-exchange normalized head outputs by q-quarter (2MB instead of an
8.4MB ReduceScatter on the projected output), and each core applies the full
W_O projection locally to its 512-row shard. Host concatenates the shards.

Attention is flash-style with transposed scores:
  sT[k, q] = K Q^T  (k on partitions), pattern = exp(sT/8) on ACT,
  AV uses stationary [v | 1] so PSUM row 64 accumulates the softmax
  denominator for free. Causality at tile granularity: upper-triangular
  tiles are skipped and diagonal tiles are column-trimmed + masked with a
  single 128x128 triangle.
All matmuls run in fp32r (full-rate reduced-precision fp32); every matmul
input is produced by an fp32r-rounding instruction chain (DMA of fp32r
tensors or engine writes to fp32r tiles).
"""

import os
import sys

sys.path.insert(0, "/opt/trn_rl_repo")

import numpy as np

# ---- problem constants (hardcoded; kernel.py must be self-contained) ----
B = 2
S = 2048
D = 1024
N_HEADS = 16
DH = 64                 # head dim
NCORES = 8
NH_CORE = N_HEADS // 4  # 4 heads per core (4-way TP x 2-way batch DP)
SCALE = 1.0 / 8.0       # 1/sqrt(64)

P = 128                 # partitions
DC = D // P             # 8 contraction chunks for the projections
KC = S // P             # 16 key chunks
QT = 512                # q tile width (free dim) for scores / AV
NQT = S // QT           # 4 q tiles
NT = 512                # moving-operand tile for projections / out-proj
GRP = 4                 # cores per batch group

_CACHE = {}


def _build():
    import concourse.bass as bass
    import concourse.tile as tile
    from concourse import bacc, mybir

    f32 = mybir.dt.float32
    MM = mybir.dt.float32r

    nc = bacc.Bacc(
        "TRN2",
        target_bir_lowering=False,
        debug=False,
        enable_asserts=False,
        num_devices=NCORES,
    )

    xt_d = nc.dram_tensor("xt", [D, S], MM, kind="ExternalInput").ap()
    wqt_d = nc.dram_tensor("wqt", [D, NH_CORE * DH], MM, kind="ExternalInput").ap()
    wkt_d = nc.dram_tensor("wkt", [D, NH_CORE * DH], MM, kind="ExternalInput").ap()
    wvt_d = nc.dram_tensor("wvt", [D, NH_CORE * DH], MM, kind="ExternalInput").ap()
    wof_d = nc.dram_tensor("wof", [N_HEADS * DH, D], MM, kind="ExternalInput").ap()
    msk_d = nc.dram_tensor("msk", [P, P], MM, kind="ExternalInput").ap()
    out_d = nc.dram_tensor("out", [S // GRP, D], f32, kind="ExternalOutput").ap()
    dbg_mode = int(os.environ.get("KERNEL_DEBUG", "0"))
    dbg_d = None
    if dbg_mode == 1:
        dbg_d = nc.dram_tensor("dbg", [2 * P, S], f32, kind="ExternalOutput").ap()
    elif dbg_mode == 2:
        dbg_d = nc.dram_tensor("dbg", [GRP * 2 * P, S], f32, kind="ExternalOutput").ap()

    Exp = mybir.ActivationFunctionType.Exp

    with tile.TileContext(nc) as tc:
        with (
            tc.tile_pool(name="const", bufs=1) as const,
            tc.tile_pool(name="work", bufs=2) as work,
            tc.tile_pool(name="ps", bufs=2, space="PSUM") as ps_pool,
            tc.tile_pool(name="attnps", bufs=1, space="PSUM") as attnps_pool,
            tc.tile_pool(name="dram", bufs=1, space="DRAM") as dram,
        ):
            xt_pool_cm = tc.tile_pool(name="xtp", bufs=1)
            xt_pool = xt_pool_cm.__enter__()
            # ---------------- input DMAs ----------------
            wq_sb = xt_pool.tile([P, DC, NH_CORE * DH], MM)
            nc.sync.dma_start(wq_sb[:], wqt_d.rearrange("(c p) n -> p c n", p=P))
            wk_sb = xt_pool.tile([P, DC, NH_CORE * DH], MM)
            nc.sync.dma_start(wk_sb[:], wkt_d.rearrange("(c p) n -> p c n", p=P))
            wv_sb = xt_pool.tile([P, DC, NH_CORE * DH], MM)
            nc.sync.dma_start(wv_sb[:], wvt_d.rearrange("(c p) n -> p c n", p=P))

            # residual^T, split per d-chunk so QKV matmuls can start early
            xt_sb = xt_pool.tile([P, DC, S], MM)
            xt_r = xt_d.rearrange("(c p) s -> p c s", p=P)
            for dc in range(DC):
                nc.sync.dma_start(xt_sb[:, dc, :], xt_r[:, dc, :])

            tri_sb = const.tile([P, P], MM)
            nc.sync.dma_start(tri_sb[:], msk_d)

            # ---------------- QKV projections ----------------
            qT = [const.tile([P, S], MM, name=f"qT{i}") for i in range(2)]
            kT = [const.tile([P, S], MM, name=f"kT{i}") for i in range(2)]
            for pr in range(2):
                for w_sb, dst in ((wq_sb, qT[pr]), (wk_sb, kT[pr])):
                    for ntile in range(S // NT):
                        pp = ps_pool.tile([P, NT], f32, name="work_ps")
                        for dc in range(DC):
                            nc.tensor.matmul(
                                pp[:],
                                w_sb[:, dc, pr * P : (pr + 1) * P],
                                xt_sb[:, dc, ntile * NT : (ntile + 1) * NT],
                                start=(dc == 0),
                                stop=(dc == DC - 1),
                            )
                        nc.scalar.copy(dst[:, ntile * NT : (ntile + 1) * NT], pp[:])

            # v in natural [k, h] layout with an appended ones column
            v_aug = [const.tile([P, KC, DH + 1], MM, name=f"vaug{h}") for h in range(NH_CORE)]
            ones_f32 = const.tile([P, DH], f32)
            nc.vector.memset(ones_f32[:], 1.0)
            ones_sb = const.tile([P, DH], MM)
            nc.scalar.copy(ones_sb[:], ones_f32[:])
            for h in range(NH_CORE):
                nc.scalar.copy(v_aug[h][:, :, DH : DH + 1], ones_f32[:, 0:KC, None])
            for pc in range(KC):
                vp = ps_pool.tile([P, NH_CORE * DH], f32, name="work_ps")
                for dc in range(DC):
                    nc.tensor.matmul(
                        vp[:],
                        xt_sb[:, dc, pc * P : (pc + 1) * P],
                        wv_sb[:, dc, :],
                        start=(dc == 0),
                        stop=(dc == DC - 1),
                    )
                for h in range(NH_CORE):
                    nc.vector.tensor_copy(
                        v_aug[h][:, pc, 0:DH],
                        vp[:, h * DH : (h + 1) * DH],
                    )

            xt_pool_cm.__exit__(None, None, None)  # frees 64KB/partition
            late_cm = tc.tile_pool(name="late", bufs=1)
            late = late_cm.__enter__()
            # full W_O (needed only after the collective; DMA it late)
            wo_sb = late.tile([P, DC, D], MM)
            nc.sync.dma_start(wo_sb[:], wof_d.rearrange("(c p) d -> p c d", p=P))

            # ---------------- attention ----------------
            # unnormalized attn_outT + denominator, per (pair, head): [65, S]
            uraw = [
                [late.tile([DH + 1, S], f32, name=f"uraw{pr}{h2}") for h2 in range(2)]
                for pr in range(2)
            ]
            deferred = []
            for pr in range(2):
                for qt in range(NQT):
                    q_sl = slice(qt * QT, (qt + 1) * QT)
                    nk = (qt + 1) * (QT // P)
                    attn_ps = [
                        attnps_pool.tile([P, QT], f32, name=f"attn{h2}") for h2 in range(2)
                    ]
                    for kb in range(nk):
                        k_sl = slice(kb * P, (kb + 1) * P)
                        ri = kb - qt * (QT // P)  # >= 0 on diagonal tiles
                        r = max(ri, 0) * P        # first valid column in this q tile
                        c_sl = slice(qt * QT + r, (qt + 1) * QT)
                        s_ps = [
                            ps_pool.tile([P, QT], f32, name=f"s{h2}", bufs=2)
                            for h2 in range(2)
                        ]
                        for h2 in range(2):
                            hb = h2 * DH
                            nc.tensor.matmul(
                                s_ps[h2][:, r:QT],
                                kT[pr][hb : hb + DH, k_sl],
                                qT[pr][hb : hb + DH, c_sl],
                                start=True,
                                stop=True,
                            )
                        for h2 in range(2):
                            h = pr * 2 + h2
                            pat = work.tile([P, QT], MM, name="pat", bufs=3)
                            nc.scalar.activation(
                                pat[:, r:QT], s_ps[h2][:, r:QT], Exp, scale=SCALE
                            )
                            if ri >= 0:
                                nc.vector.tensor_mul(
                                    pat[:, r : r + P], pat[:, r : r + P], tri_sb[:]
                                )
                            nc.tensor.matmul(
                                attn_ps[h2][0 : DH + 1, r:QT],
                                v_aug[h][:, kb, :],
                                pat[:, r:QT],
                                start=(kb == 0),
                                stop=(kb == nk - 1),
                            )
                    for h2 in range(2):
                        # evacuate accumulator quickly (frees PSUM for next qt),
                        # reciprocal of the denominator row in place
                        nc.scalar.copy(uraw[pr][h2][:, q_sl], attn_ps[h2][0 : DH + 1, :])
                        nc.vector.reciprocal(
                            uraw[pr][h2][DH : DH + 1, q_sl],
                            uraw[pr][h2][DH : DH + 1, q_sl],
                        )
                        deferred.append((pr, qt, h2))

            # ---------------- normalize + AllGather ----------------
            # cc_in rows [pr*128 + h2*64 : +64] = normalized attn_outT of head
            # (pr*2+h2), all q. After AllGather, cc_out rows [i*256:(i+1)*256]
            # = core i's heads -> global head order matches W_O rows.
            cc_in = dram.tile([2 * P, S], MM)
            cc_out = dram.tile([GRP * 2 * P, S], MM)
            for pr, qt, h2 in deferred:
                q_sl = slice(qt * QT, (qt + 1) * QT)
                rb_ps = ps_pool.tile([DH, QT], f32, name="work_ps")
                nc.tensor.matmul(
                    rb_ps[:],
                    ones_f32[DH : DH + 1, :],
                    uraw[pr][h2][DH : DH + 1, q_sl],
                    start=True,
                    stop=True,
                )
                u_n = work.tile([DH, QT], MM, name="u_n", bufs=4)
                nc.vector.tensor_mul(u_n[:], uraw[pr][h2][0:DH, q_sl], rb_ps[:])
                row = pr * P + h2 * DH
                nc.sync.dma_start(cc_in[row : row + DH, q_sl], u_n[:])

            nc.gpsimd.collective_compute(
                "AllGather",
                mybir.AluOpType.bypass,
                replica_groups=[[0, 1, 2, 3], [4, 5, 6, 7]],
                ins=[cc_in.opt()],
                outs=[cc_out.opt()],
            )

            if dbg_mode == 1:
                nc.sync.dma_start(dbg_d, cc_in[:].bitcast(f32))
            elif dbg_mode == 2:
                nc.sync.dma_start(dbg_d, cc_out[:].bitcast(f32))
            # ------------- local W_O projection on own q-quarter -------------
            # q-quarter = group rank = partition_id % 4 (dynamic column slice)
            pid = nc.partition_id()
            qoff = nc.snap((pid % GRP) * QT)
            attR = late.tile([P, DC, QT], MM)
            cc_r = cc_out[:].rearrange("(c p) q -> p c q", p=P)
            nc.sync.dma_start(attR[:], cc_r[:, :, bass.ds(qoff, QT)])
            for pc in range(QT // P):
                osb = work.tile([P, D], f32, name="osb", bufs=2)
                for dt_ in range(D // NT):
                    op = ps_pool.tile([P, NT], f32, name="work_ps")
                    for c in range(DC):
                        nc.tensor.matmul(
                            op[:],
                            attR[:, c, pc * P : (pc + 1) * P],
                            wo_sb[:, c, dt_ * NT : (dt_ + 1) * NT],
                            start=(c == 0),
                            stop=(c == DC - 1),
                        )
                    nc.scalar.copy(osb[:, dt_ * NT : (dt_ + 1) * NT], op[:])
                nc.sync.dma_start(out_d[pc * P : (pc + 1) * P, :], osb[:])
            late_cm.__exit__(None, None, None)

    nc.compile()
    return nc


def _get_nc():
    if "nc" not in _CACHE:
        _CACHE["nc"] = _build()
    return _CACHE["nc"]


def _tri():
    k = np.arange(P)[:, None]
    q = np.arange(P)[None, :]
    return (q >= k).astype(np.float32)


def _ensure_ntff_hook():
    """Register the axon NTFF profile hook (missing antenv.axon_hooks shim)."""
    import types

    try:
        from antenv.axon_hooks import get_axon_ntff_profile_hook  # noqa: F401

        return
    except ImportError:
        pass
    import antenv

    if "/root/.axon_site" not in sys.path:
        sys.path.insert(0, "/root/.axon_site")
    from trn_agent_boot.trn_boot import _ntff_profile_via_ctypes

    hook = _ntff_profile_via_ctypes("/opt/axon/libaxon_pjrt.so")
    mod = types.ModuleType("antenv.axon_hooks")
    mod.get_axon_ntff_profile_hook = lambda: hook
    mod.set_axon_ntff_profile_hook = lambda h: None
    sys.modules["antenv.axon_hooks"] = mod
    antenv.axon_hooks = mod


def kernel(residual, W_Q, W_K, W_V, W_O):
    from concourse.bass_utils import run_bass_kernel_spmd

    if int(os.environ.get("KERNEL_TRACE", "0")):
        _ensure_ntff_hook()

    residual = np.ascontiguousarray(np.asarray(residual), np.float32)
    W_Q = np.ascontiguousarray(np.asarray(W_Q), np.float32)
    W_K = np.ascontiguousarray(np.asarray(W_K), np.float32)
    W_V = np.ascontiguousarray(np.asarray(W_V), np.float32)
    W_O = np.ascontiguousarray(np.asarray(W_O), np.float32)

    nc = _get_nc()
    tri = _tri()
    wof = np.ascontiguousarray(W_O.reshape(N_HEADS * DH, D))
    in_maps = []
    for c in range(NCORES):
        b, g = divmod(c, GRP)
        hs = slice(g * NH_CORE, (g + 1) * NH_CORE)
        in_maps.append(
            {
                "xt": np.ascontiguousarray(residual[b].T),
                "wqt": np.ascontiguousarray(
                    W_Q[hs].transpose(2, 0, 1).reshape(D, NH_CORE * DH)
                ),
                "wkt": np.ascontiguousarray(
                    W_K[hs].transpose(2, 0, 1).reshape(D, NH_CORE * DH)
                ),
                "wvt": np.ascontiguousarray(
                    W_V[hs].transpose(2, 0, 1).reshape(D, NH_CORE * DH)
                ),
                "wof": wof,
                "msk": tri,
            }
        )

    res = run_bass_kernel_spmd(
        nc,
        in_maps,
        core_ids=list(range(NCORES)),
        trace=bool(int(os.environ.get("KERNEL_TRACE", "0"))),
        trace_cores=[0] if int(os.environ.get("KERNEL_TRACE", "0")) else None,
    )
    _CACHE["last_results"] = res

    out = np.empty((B, S, D), np.float32)
    for b in range(B):
        out[b] = np.concatenate(
            [res.results[b * GRP + r]["out"] for r in range(GRP)], axis=0
        )
    return out
